# revision 1
# baseline (speedup 1.0000x reference)
"""Single-head causal attention (B=256, T=256, C=1024, D=64) on 8 TRN2 NeuronCores.

Strategy: data-parallel over batch (32 batches/core). Host pre-transposes x to
x^T [B, C, T] bf16 so the contraction dim C lands on SBUF partitions with fully
contiguous DMA, eliminating every on-device transpose:

  per batch b (all layouts partition-major):
    qk^T [128,T]  = [Wq|Wk]^T @ x_b^T        (8 accumulating matmuls, K=128)
    sc^T [S,T]    = k @ q^T                   (2 matmuls, K=64; the fully-masked
                                               (s>=128, t<128) quadrant is skipped)
    e^T           = exp(sc^T / 32)            (one ScalarE op over [128, 384])
    e^T quadrants causal-zeroed via GpSimd affine_select (only the 2 diagonal
                                               [128,128] quadrants need masking)
    v    [S,D]    = (x_b^T)^T @ Wv            (16 matmuls, xT-stationary)
    o'   [T,2,D+1]= e^T.T @ [v | 1]           (numerator and softmax denominator
                                               fused into one accumulation)
    out  [T,D]    = o' * (1/den)              (VectorE reciprocal + broadcast mult)

The final stage of batch b is emitted during batch b+1 (software pipelining) so
the in-order PE stream never stalls on the exp/mask chain. Softmax needs no
max-subtraction: scores/32 ~ N(0, 0.25^2), |max| < ~1.6, so exp never overflows.
"""

import numpy as np
import ml_dtypes

import concourse.bacc as bacc
import concourse.mybir as mybir
import concourse.tile as tile
from concourse.bass_utils import run_bass_kernel_spmd

B, T, C, D = 256, 256, 1024, 64
NCORES = 8
BPC = B // NCORES  # batches per core
CCH = C // 128  # contraction chunks
SCALE = float(C) ** -0.5

BF16 = mybir.dt.bfloat16
F32 = mybir.dt.float32

TRACE = False
LAST_RESULT = None


def _build(
    xp_bufs=6,
    ep_bufs=4,
    vp_bufs=4,
    qk_ps_bufs=2,
    sc_ps_bufs=3,
    v_ps_bufs=2,
    o_ps_bufs=1,
    alt_rings=False,
    split_xt=True,
    xt_ways=2,
    split_out=False,
    early_final=False,
):
    nc = bacc.Bacc(
        "TRN2", target_bir_lowering=False, debug=False, num_devices=NCORES
    )
    xt = nc.dram_tensor("xt", [BPC, C, T], BF16, kind="ExternalInput")
    wqk = nc.dram_tensor("wqk", [C, 128], BF16, kind="ExternalInput")
    wv = nc.dram_tensor("wv", [C, D], BF16, kind="ExternalInput")
    out = nc.dram_tensor("out", [BPC, T, D], F32, kind="ExternalOutput")

    with tile.TileContext(nc) as tc:
        with (
            tc.tile_pool(name="singles", bufs=1) as singles,
            tc.tile_pool(name="xp", bufs=xp_bufs) as xp,
            tc.tile_pool(name="sbp", bufs=3) as sbp,
            tc.tile_pool(name="ep", bufs=ep_bufs) as ep,
            tc.tile_pool(name="vp", bufs=vp_bufs) as vp,
            tc.tile_pool(name="outp", bufs=4) as outp,
            tc.tile_pool(name="qk_ps", bufs=qk_ps_bufs, space="PSUM") as qk_psp,
            tc.tile_pool(name="sc_ps", bufs=sc_ps_bufs, space="PSUM") as sc_psp,
            tc.tile_pool(name="v_ps", bufs=v_ps_bufs, space="PSUM") as v_psp,
            tc.tile_pool(name="o_ps", bufs=o_ps_bufs, space="PSUM") as o_psp,
        ):
            wqk_sb = singles.tile([128, CCH, 128], BF16)
            nc.sync.dma_start(wqk_sb, wqk[:].rearrange("(c p) m -> p c m", p=128))
            wv_sb = singles.tile([128, CCH, D], BF16)
            nc.sync.dma_start(wv_sb, wv[:].rearrange("(c p) m -> p c m", p=128))

            def final_stage(b, expT, v_sb):
                """Final matmuls + softmax normalization + out-DMA for batch b.

                Emitted one batch late so the PE work here never waits on the
                (ACT exp / Pool mask) chain of the same batch.
                """
                # o2[:, tt, :] = [num | den] for t-tile tt; one PSUM bank.
                o2 = o_psp.tile([128, 2, D + 1], F32, tag="o_ps")
                nc.tensor.matmul(
                    o2[:, 0],
                    lhsT=expT[:, 0:128],
                    rhs=v_sb[:, 0],
                    start=True,
                    stop=True,
                )
                nc.tensor.matmul(
                    o2[:, 1],
                    lhsT=expT[:, 128:256],
                    rhs=v_sb[:, 0],
                    start=True,
                    stop=False,
                )
                nc.tensor.matmul(
                    o2[:, 1],
                    lhsT=expT[:, 256:384],
                    rhs=v_sb[:, 1],
                    start=False,
                    stop=True,
                )
                out_v = out[b].rearrange("(tt p) d -> p tt d", p=128)
                if split_out:
                    # normalize + store per t-tile so the first out-DMA is
                    # ready as soon as tt=0's matmul lands
                    for tt in range(2):
                        recip = outp.tile([128, 1], F32, tag="recip")
                        nc.vector.reciprocal(recip, o2[:, tt, D : D + 1])
                        o_sb = outp.tile([128, D], F32, tag="o_sb")
                        nc.vector.tensor_scalar_mul(o_sb, o2[:, tt, 0:D], recip)
                        nc.scalar.dma_start(out_v[:, tt], o_sb)
                else:
                    recip = outp.tile([128, 2], F32, tag="recip")
                    nc.vector.reciprocal(recip, o2[:, :, D])
                    o_sb = outp.tile([128, 2, D], F32, tag="o_sb")
                    nc.vector.tensor_tensor(
                        o_sb,
                        o2[:, :, 0:D],
                        recip[:, :, None].to_broadcast((128, 2, D)),
                        mybir.AluOpType.mult,
                    )
                    nc.scalar.dma_start(out_v, o_sb)

            prev = None
            for b in range(BPC):
                xt_t = xp.tile([128, CCH, T], BF16, tag="xt")
                dma_eng = nc.scalar if (alt_rings and b % 2) else nc.sync
                xt_src = xt[b].rearrange("(c p) t -> p c t", p=128)
                if split_xt:
                    w = CCH // xt_ways
                    for i in range(xt_ways):
                        dma_eng.dma_start(
                            xt_t[:, i * w : (i + 1) * w],
                            xt_src[:, i * w : (i + 1) * w],
                        )
                else:
                    dma_eng.dma_start(xt_t, xt_src)

                qk_ps = qk_psp.tile([128, T], F32, tag="qk")
                for c in range(CCH):
                    nc.tensor.matmul(
                        qk_ps,
                        lhsT=wqk_sb[:, c],
                        rhs=xt_t[:, c],
                        start=(c == 0),
                        stop=(c == CCH - 1),
                    )

                if early_final and prev is not None:
                    # emit last batch's finale right after this batch's qk
                    # matmuls: its out-DMA issue isn't queued behind this
                    # batch's exp on ACT, and its PE matmuls widen the window
                    # for the DVE q/k copies before the scores matmuls.
                    final_stage(*prev)

                # v projection (independent of scores chain; keeps PE busy
                # while DVE copies q/k out of PSUM)
                v_sb = vp.tile([128, 2, D + 1], BF16, tag="v")
                for st in range(2):
                    v_ps = v_psp.tile([128, D], F32, tag="v_ps")
                    for c in range(CCH):
                        nc.tensor.matmul(
                            v_ps,
                            lhsT=xt_t[:, c, st * 128 : (st + 1) * 128],
                            rhs=wv_sb[:, c],
                            start=(c == 0),
                            stop=(c == CCH - 1),
                        )
                    nc.scalar.copy(v_sb[:, st, 0:D], v_ps)
                    nc.gpsimd.memset(v_sb[:, st, D : D + 1], 1.0)

                q_sb = sbp.tile([64, T], BF16, tag="q_sb")
                nc.vector.tensor_copy(q_sb, qk_ps[0:64, :])
                k_sb = sbp.tile([64, T], BF16, tag="k_sb")
                nc.vector.tensor_copy(k_sb, qk_ps[64:128, :])

                # scores^T, packed in one PSUM bank [128, 384]:
                # cols 0:256   = st=0 (all t)
                # cols 256:384 = st=1, t in [128, 256)
                # (the (st=1, t<128) quadrant is fully causal-masked, skipped)
                sc_ps = sc_psp.tile([128, 3 * 128], F32, tag="sc")
                nc.tensor.matmul(
                    sc_ps[:, 0:T],
                    lhsT=k_sb[:, 0:128],
                    rhs=q_sb[:],
                    start=True,
                    stop=True,
                )
                nc.tensor.matmul(
                    sc_ps[:, T : T + 128],
                    lhsT=k_sb[:, 128:T],
                    rhs=q_sb[:, 128:T],
                    start=True,
                    stop=True,
                )
                expT = ep.tile([128, 3 * 128], BF16, tag="expT")
                nc.scalar.activation(
                    expT,
                    sc_ps,
                    func=mybir.ActivationFunctionType.Exp,
                    scale=SCALE,
                )
                # triangular causal mask on the two diagonal quadrants
                # (the (st=0, t>=128) quadrant is fully unmasked)
                for quad in (0, 256):
                    nc.gpsimd.affine_select(
                        out=expT[:, quad : quad + 128],
                        in_=expT[:, quad : quad + 128],
                        compare_op=mybir.AluOpType.is_ge,
                        fill=0.0,
                        base=0,
                        pattern=[[1, 128]],
                        channel_multiplier=-1,
                    )

                if not early_final and prev is not None:
                    final_stage(*prev)
                prev = (b, expT, v_sb)
            final_stage(*prev)
    nc.compile()
    return nc


def kernel(x: np.ndarray, Wq: np.ndarray, Wk: np.ndarray, Wv: np.ndarray) -> np.ndarray:
    global LAST_RESULT
    x = np.asarray(x, dtype=np.float32)
    Wq = np.asarray(Wq, dtype=np.float32)
    Wk = np.asarray(Wk, dtype=np.float32)
    Wv = np.asarray(Wv, dtype=np.float32)
    xt = np.ascontiguousarray(np.transpose(x, (0, 2, 1))).astype(ml_dtypes.bfloat16)
    wqk = np.concatenate([Wq, Wk], axis=1).astype(ml_dtypes.bfloat16)
    wv = np.ascontiguousarray(Wv).astype(ml_dtypes.bfloat16)

    nc = _build()
    in_maps = [
        {"xt": xt[i * BPC : (i + 1) * BPC], "wqk": wqk, "wv": wv}
        for i in range(NCORES)
    ]
    res = run_bass_kernel_spmd(
        nc, in_maps, core_ids=list(range(NCORES)), trace=TRACE
    )
    LAST_RESULT = res
    out = np.concatenate([r["out"] for r in res.results], axis=0)
    return out


if __name__ == "__main__":
    x = np.random.randn(B, T, C).astype(np.float32)
    Wq = np.random.randn(C, D).astype(np.float32) * (C**-0.5)
    Wk = np.random.randn(C, D).astype(np.float32) * (C**-0.5)
    Wv = np.random.randn(C, D).astype(np.float32) * (C**-0.5)
    o = kernel(x, Wq, Wk, Wv)
    print(o.shape, o.dtype)



# revision 4
# speedup vs baseline: 1.1937x; 1.1937x over previous
"""Single-head causal attention (B=256, T=256, C=1024, D=64) on 8 TRN2 NeuronCores.

Data-parallel over batch (32 batches/core). The schedule is built to be
DMA-bandwidth-bound (the per-core x traffic is the roofline) with every other
engine holding slack:

  * x ships as an fp8-e4m3 hi/lo pair (x = xh + xl, xl the unscaled residual)
    pre-transposed to partition-major [C, T]. Same bytes as bf16, but every
    projection matmul runs in DoubleRow fp8 mode (4x bf16 throughput):
        q|k^T [128,T] = sum of 3 terms  xh@wh + xl@wh + xh@wl   (12 matmuls)
        v     [T,D]   = same 3 terms, xT-stationary             (24 matmuls)
    Weights are host-prescaled by 8 so their fp8 split stays in the normal
    range; the 8x comes out in the softmax scale and the fused denominator.
  * scores^T = k @ q^T in bf16 from a single [128,T] PSUM->SBUF copy (k rows
    sit at partitions 64:128; tile_position=(0,0) passed explicitly).
  * exp via ACT (scale absorbs C**-0.5 / 8**2), causal mask via GpSimd
    affine_select on the two diagonal quadrants only.
  * o' [T, 2, D+1] = e^T.T @ [v | 8] fuses numerator + denominator; DVE
    normalizes into a bf16 2-batch staging tile (host upcasts to f32).
  * DMA: one 1MB load per 2 batches (8KB/partition contiguous), one 64KB
    store per 2 batches, one packed weight load -- 34 DMA instructions total,
    all with >=512B descriptors (no sub-512B latency penalty).
"""

import numpy as np
import ml_dtypes

import concourse.bacc as bacc
import concourse.mybir as mybir
import concourse.tile as tile
from concourse.bass_utils import run_bass_kernel_spmd

B, T, C, D = 256, 256, 1024, 64
NCORES = 8
BPC = B // NCORES  # batches per core
NSB = BPC // 2  # superbatches (2 batches per DMA)
CCH = C // 128  # contraction chunks
NCP = CCH // 2  # chunk pairs (DoubleRow contracts 2 chunks/instruction)
WS = 8.0  # host weight prescale; keeps fp8 weight splits in normal range
SCALE = float(C) ** -0.5

BF16 = mybir.dt.bfloat16
F32 = mybir.dt.float32
F8 = mybir.dt.float8e4
E4M3 = ml_dtypes.float8_e4m3
DR = mybir.MatmulPerfMode.DoubleRow

TRACE = False
LAST_RESULT = None

# (x half, w half) term order: hi@hi, lo@hi, hi@lo
TERMS = ((0, 0), (1, 0), (0, 1))


def _build(pf=3, qk_terms=3, v_terms=3):
    nc = bacc.Bacc(
        "TRN2", target_bir_lowering=False, debug=False, num_devices=NCORES
    )
    # [sb, partition, batch-in-sb, hi/lo, chunk, t]
    xhl = nc.dram_tensor("xhl", [NSB, 128, 2, 2, CCH, T], F8, kind="ExternalInput")
    # [partition, chunk, hi/lo, 192] : cols 0:128 = [Wq|Wk]*8, 128:192 = Wv*8
    wall = nc.dram_tensor("wall", [128, CCH, 2, 192], F8, kind="ExternalInput")
    # [sb, partition, batch-in-sb, t-tile, d]
    out = nc.dram_tensor("out", [NSB, 128, 2, 2, D], BF16, kind="ExternalOutput")

    with tile.TileContext(nc) as tc:
        with (
            tc.tile_pool(name="singles", bufs=1) as singles,
            tc.tile_pool(name="xp", bufs=pf + 1) as xp,
            tc.tile_pool(name="sbp", bufs=3) as sbp,
            tc.tile_pool(name="ep", bufs=4) as ep,
            tc.tile_pool(name="vp", bufs=4) as vp,
            tc.tile_pool(name="stp", bufs=2) as stp,
            tc.tile_pool(name="rp", bufs=2) as rp,
            tc.tile_pool(name="qk_ps", bufs=2, space="PSUM") as qk_psp,
            tc.tile_pool(name="sc_ps", bufs=3, space="PSUM") as sc_psp,
            tc.tile_pool(name="v_ps", bufs=2, space="PSUM") as v_psp,
            tc.tile_pool(name="o_ps", bufs=1, space="PSUM") as o_psp,
        ):
            w_sb = singles.tile([128, CCH, 2, 192], F8)
            nc.sync.dma_start(w_sb, wall[:])

            xt_tiles = {}

            def load_sb(k):
                t = xp.tile([128, 2, 2, CCH, T], F8, tag="xt")
                nc.sync.dma_start(t, xhl[k])
                xt_tiles[k] = t

            stages = {}

            def final_stage(b, expT, v_sb):
                """o' matmuls + softmax normalization for batch b (emitted one
                batch late so PE never waits on the exp/mask chain)."""
                sb, bi = divmod(b, 2)
                o2 = o_psp.tile([128, 2, D + 1], F32, tag="o_ps")
                nc.tensor.matmul(
                    o2[:, 0], lhsT=expT[:, 0:128], rhs=v_sb[:, 0],
                    start=True, stop=True,
                )
                nc.tensor.matmul(
                    o2[:, 1], lhsT=expT[:, 128:256], rhs=v_sb[:, 0],
                    start=True, stop=False,
                )
                nc.tensor.matmul(
                    o2[:, 1], lhsT=expT[:, 256:384], rhs=v_sb[:, 1],
                    start=False, stop=True,
                )
                if bi == 0:
                    stages[sb] = stp.tile(
                        [128, 2, 2, D], BF16, tag="stage", name="stage"
                    )
                recip = rp.tile([128, 2], F32, tag="recip")
                nc.vector.reciprocal(recip, o2[:, :, D])
                nc.vector.tensor_tensor(
                    stages[sb][:, bi],
                    o2[:, :, 0:D],
                    recip[:, :, None].to_broadcast((128, 2, D)),
                    mybir.AluOpType.mult,
                )

            for k in range(min(pf, NSB)):
                load_sb(k)

            prev = None
            for b in range(BPC):
                sb, bi = divmod(b, 2)
                if bi == 0 and sb + pf < NSB:
                    load_sb(sb + pf)
                if bi == 1 and sb >= 1:
                    nc.sync.dma_start(out[sb - 1], stages.pop(sb - 1))
                xt = xt_tiles[sb]

                # q|k projection: 3 DoubleRow terms, one PSUM group
                qk_ps = qk_psp.tile([128, T], F32, tag="qk")
                n = qk_terms * NCP
                i = 0
                for xh_, wh_ in TERMS[:qk_terms]:
                    for cp in range(NCP):
                        nc.tensor.matmul(
                            qk_ps,
                            lhsT=w_sb[:, 2 * cp : 2 * cp + 2, wh_, 0:128],
                            rhs=xt[:, bi, xh_, 2 * cp : 2 * cp + 2, :],
                            start=(i == 0),
                            stop=(i == n - 1),
                            perf_mode=DR,
                        )
                        i += 1

                if prev is not None:
                    final_stage(*prev)

                # v projection (keeps PE busy while DVE copies q/k)
                v_sb = vp.tile([128, 2, D + 1], BF16, tag="v")
                n = v_terms * NCP
                for st in range(2):
                    v_ps = v_psp.tile([128, D], F32, tag="v_ps")
                    i = 0
                    for xh_, wh_ in TERMS[:v_terms]:
                        for cp in range(NCP):
                            nc.tensor.matmul(
                                v_ps,
                                lhsT=xt[
                                    :, bi, xh_, 2 * cp : 2 * cp + 2,
                                    st * 128 : (st + 1) * 128,
                                ],
                                rhs=w_sb[:, 2 * cp : 2 * cp + 2, wh_, 128:192],
                                start=(i == 0),
                                stop=(i == n - 1),
                                perf_mode=DR,
                            )
                            i += 1
                    nc.scalar.copy(v_sb[:, st, 0:D], v_ps)
                    nc.gpsimd.memset(v_sb[:, st, D : D + 1], WS)

                q_sb = sbp.tile([64, T], BF16, tag="q_sb")
                nc.vector.tensor_copy(q_sb, qk_ps[0:64, :])
                k_sb = sbp.tile([64, T], BF16, tag="k_sb")
                nc.vector.tensor_copy(k_sb, qk_ps[64:128, :])

                # scores^T packed [128, 384]: cols 0:256 = (s<128, all t),
                # 256:384 = (s>=128, t>=128); (s>=128, t<128) fully masked
                sc_ps = sc_psp.tile([128, 3 * 128], F32, tag="sc")
                nc.tensor.matmul(
                    sc_ps[:, 0:T],
                    lhsT=k_sb[:, 0:128],
                    rhs=q_sb[:],
                    start=True, stop=True,
                )
                nc.tensor.matmul(
                    sc_ps[:, T : T + 128],
                    lhsT=k_sb[:, 128:T],
                    rhs=q_sb[:, 128:T],
                    start=True, stop=True,
                )
                expT = ep.tile([128, 3 * 128], BF16, tag="expT")
                nc.scalar.activation(
                    expT, sc_ps,
                    func=mybir.ActivationFunctionType.Exp,
                    scale=SCALE / (WS * WS),
                )
                for quad in (0, 256):
                    nc.gpsimd.affine_select(
                        out=expT[:, quad : quad + 128],
                        in_=expT[:, quad : quad + 128],
                        compare_op=mybir.AluOpType.is_ge,
                        fill=0.0,
                        base=0,
                        pattern=[[1, 128]],
                        channel_multiplier=-1,
                    )
                prev = (b, expT, v_sb)

            final_stage(*prev)
            nc.sync.dma_start(out[NSB - 1], stages.pop(NSB - 1))
    nc.compile()
    return nc


def _pack_inputs(x, Wq, Wk, Wv):
    """Host-side layout/dtype prep: per-core [NSB,128,2,2,CCH,T] fp8 hi/lo x
    and the shared packed weight blob."""
    xt = np.ascontiguousarray(np.transpose(x, (0, 2, 1)))  # [B, C, T] f32
    xh = xt.astype(E4M3)
    xl = (xt - xh.astype(np.float32)).astype(E4M3)
    # [B, C, T] -> [B//2, 2, CCH, 128, T] -> stack hl -> [NSB*8, 128, 2, 2, CCH, T]
    def pack(a):
        return a.reshape(B // 2, 2, CCH, 128, T)
    ph, pl = pack(xh), pack(xl)
    xhl = np.stack([ph, pl], axis=2)  # [B//2, 2, 2, CCH, 128, T]
    xhl = np.ascontiguousarray(xhl.transpose(0, 4, 1, 2, 3, 5))

    wcat = np.concatenate([Wq, Wk, Wv], axis=1) * WS  # [C, 192]
    wh = wcat.astype(E4M3)
    wl = (wcat - wh.astype(np.float32)).astype(E4M3)
    # [C, 192] -> [CCH, 128, 192] -> [128, CCH, 2, 192]
    wall = np.stack(
        [wh.reshape(CCH, 128, 192), wl.reshape(CCH, 128, 192)], axis=2
    ).transpose(1, 0, 2, 3)
    return np.ascontiguousarray(xhl), np.ascontiguousarray(wall)


def kernel(x: np.ndarray, Wq: np.ndarray, Wk: np.ndarray, Wv: np.ndarray) -> np.ndarray:
    global LAST_RESULT
    x = np.asarray(x, dtype=np.float32)
    Wq = np.asarray(Wq, dtype=np.float32)
    Wk = np.asarray(Wk, dtype=np.float32)
    Wv = np.asarray(Wv, dtype=np.float32)

    xhl, wall = _pack_inputs(x, Wq, Wk, Wv)

    nc = _build()
    in_maps = [
        {"xhl": xhl[i * NSB : (i + 1) * NSB], "wall": wall}
        for i in range(NCORES)
    ]
    res = run_bass_kernel_spmd(
        nc, in_maps, core_ids=list(range(NCORES)), trace=TRACE
    )
    LAST_RESULT = res
    # [NSB, 128, 2, 2, D] -> [NSB, 2, 2, 128, D] -> [BPC, T, D]
    outs = [
        np.ascontiguousarray(r["out"].transpose(0, 2, 3, 1, 4))
        .reshape(BPC, T, D)
        .astype(np.float32)
        for r in res.results
    ]
    return np.concatenate(outs, axis=0)


if __name__ == "__main__":
    x = np.random.randn(B, T, C).astype(np.float32)
    Wq = np.random.randn(C, D).astype(np.float32) * (C**-0.5)
    Wk = np.random.randn(C, D).astype(np.float32) * (C**-0.5)
    Wv = np.random.randn(C, D).astype(np.float32) * (C**-0.5)
    o = kernel(x, Wq, Wk, Wv)
    print(o.shape, o.dtype)


# revision 32
# speedup vs baseline: 1.2291x; 1.0296x over previous
"""Single-head causal attention (B=256, T=256, C=1024, D=64) on 8 TRN2 NeuronCores.

Data-parallel over batch (32 batches/core). The schedule is built to be
DMA-bandwidth-bound (the per-core x traffic is the roofline) with every other
engine holding slack:

  * x ships as an fp8-e4m3 hi/lo pair (x = xh + xl, xl the unscaled residual)
    pre-transposed to partition-major [C, T]. Same bytes as bf16, but every
    projection matmul runs in DoubleRow fp8 mode (4x bf16 throughput):
        q|k^T [128,T] = sum of 3 terms  xh@wh + xl@wh + xh@wl   (12 matmuls)
        v     [T,D]   = same 3 terms, xT-stationary             (24 matmuls)
    Weights are host-prescaled by 8 so their fp8 split stays in the normal
    range; the 8x comes out in the softmax scale and the fused denominator.
  * scores^T = k @ q^T in bf16 from a single [128,T] PSUM->SBUF copy (k rows
    sit at partitions 64:128; tile_position=(0,0) passed explicitly).
  * exp via ACT (scale absorbs C**-0.5 / 8**2), causal mask via GpSimd
    affine_select on the two diagonal quadrants only.
  * o' [T, 2, D+1] = e^T.T @ [v | 8] fuses numerator + denominator; DVE
    normalizes into a bf16 2-batch staging tile (host upcasts to f32).
  * DMA: one 1MB load per 2 batches (8KB/partition contiguous), one 64KB
    store per 2 batches, one packed weight load -- 34 DMA instructions total,
    all with >=512B descriptors (no sub-512B latency penalty).
"""

import numpy as np
import ml_dtypes

import concourse.bacc as bacc
import concourse.mybir as mybir
import concourse.tile as tile
from concourse.bass_utils import run_bass_kernel_spmd

B, T, C, D = 256, 256, 1024, 64
NCORES = 8
BPC = B // NCORES  # batches per core
NSB = BPC // 2  # superbatches (2 batches per DMA)
CCH = C // 128  # contraction chunks
NCP = CCH // 2  # chunk pairs (DoubleRow contracts 2 chunks/instruction)
WS = 8.0  # host weight prescale; keeps fp8 weight splits in normal range
SCALE = float(C) ** -0.5

BF16 = mybir.dt.bfloat16
F32 = mybir.dt.float32
F8 = mybir.dt.float8e4
E4M3 = ml_dtypes.float8_e4m3
DR = mybir.MatmulPerfMode.DoubleRow

TRACE = False
LAST_RESULT = None

# (x half, w half) term order: hi@hi, lo@hi, hi@lo
TERMS = ((0, 0), (1, 0), (0, 1))


def _build(pf=2, qk_terms=2, v_terms=3, mask_eng='affine', vcopy_eng='act'):
    nc = bacc.Bacc(
        "TRN2", target_bir_lowering=False, debug=False, num_devices=NCORES
    )
    # [sb, partition, batch-in-sb, hi/lo, chunk, t]
    xhl = nc.dram_tensor("xhl", [NSB, 128, 2, 2, CCH, T], F8, kind="ExternalInput")
    # [partition, chunk, hi/lo, 192] : cols 0:128 = [Wq|Wk]*8, 128:192 = Wv*8
    wall = nc.dram_tensor("wall", [128, CCH, 2, 192], F8, kind="ExternalInput")
    # [sb, partition, batch-in-sb, t-tile, d]
    out = nc.dram_tensor("out", [NSB, 128, 2, 2, D], BF16, kind="ExternalOutput")

    with tile.TileContext(nc) as tc:
        with (
            tc.tile_pool(name="singles", bufs=1) as singles,
            tc.tile_pool(name="xp", bufs=pf + 1) as xp,
            tc.tile_pool(name="sbp", bufs=3) as sbp,
            tc.tile_pool(name="ep", bufs=8) as ep,
            tc.tile_pool(name="vp", bufs=6) as vp,
            tc.tile_pool(name="stp", bufs=5) as stp,
            tc.tile_pool(name="rp", bufs=2) as rp,
            tc.tile_pool(name="qk_ps", bufs=2, space="PSUM") as qk_psp,
            tc.tile_pool(name="sc_ps", bufs=2, space="PSUM") as sc_psp,
            tc.tile_pool(name="v_ps", bufs=2, space="PSUM") as v_psp,
            tc.tile_pool(name="o_ps", bufs=2, space="PSUM") as o_psp,
        ):
            w_sb = singles.tile([128, CCH, 2, 192], F8)
            nc.sync.dma_start(w_sb, wall[:])

            # causal triangle (1 where s <= t within a 128-tile) built once;
            # masking is then a tensor-tensor multiply on any engine
            tri = singles.tile([128, 128], BF16)
            nc.gpsimd.memset(tri, 1.0)
            nc.gpsimd.affine_select(
                out=tri, in_=tri,
                compare_op=mybir.AluOpType.is_ge,
                fill=0.0, base=0, pattern=[[1, 128]], channel_multiplier=-1,
            )

            xt_tiles = {}

            def load_sb(k, split=False):
                t = xp.tile([128, 2, 2, CCH, T], F8, tag="xt")
                if split:
                    # per-batch halves: first batch's data (and compute)
                    # lands ~1.5us earlier at the pipeline head/tail
                    nc.sync.dma_start(t[:, 0], xhl[k][:, 0])
                    nc.sync.dma_start(t[:, 1], xhl[k][:, 1])
                else:
                    nc.sync.dma_start(t, xhl[k])
                xt_tiles[k] = t

            stages = {}

            def final_stage(sb, expT0, expT1, v_sb):
                """o' matmuls + softmax normalization for superbatch sb
                (emitted two superbatches late)."""
                o2 = o_psp.tile([128, 2, 2, D + 1], F32, tag="o_ps")
                for bi, expT in ((0, expT0), (1, expT1)):
                    nc.tensor.matmul(
                        o2[:, bi, 0], lhsT=expT[:, 0:128], rhs=v_sb[:, bi, 0],
                        start=True, stop=True,
                    )
                    nc.tensor.matmul(
                        o2[:, bi, 1], lhsT=expT[:, 128:256], rhs=v_sb[:, bi, 0],
                        start=True, stop=False,
                    )
                    nc.tensor.matmul(
                        o2[:, bi, 1], lhsT=expT[:, 256:384], rhs=v_sb[:, bi, 1],
                        start=False, stop=True,
                    )
                stages[sb] = stp.tile(
                    [128, 2, 2, D], BF16, tag="stage", name="stage"
                )
                # recip to SBUF first: engines may read only ONE PSUM
                # operand per instruction, and Pool can't read PSUM at all
                recip = rp.tile([128, 2, 2], F32, tag="recip")
                nc.vector.reciprocal(recip, o2[:, :, :, D])
                nc.vector.tensor_tensor(
                    stages[sb],
                    o2[:, :, :, 0:D],
                    recip[:, :, :, None].to_broadcast((128, 2, 2, D)),
                    mybir.AluOpType.mult,
                )

            def scores_stage(sb, q_sb, k_sb, v_sb):
                """scores^T + exp + causal mask for both batches of sb
                (emitted one superbatch late)."""
                expTs = []
                for bi in range(2):
                    # scores^T packed [128, 384]: cols 0:256 = (s<128, all t),
                    # 256:384 = (s>=128, t>=128); (s>=128, t<128) fully masked
                    sc_ps = sc_psp.tile([128, 3 * 128], F32, tag="sc")
                    nc.tensor.matmul(
                        sc_ps[:, 0:T],
                        lhsT=k_sb[:, bi, 0:128],
                        rhs=q_sb[:, bi],
                        start=True, stop=True,
                    )
                    nc.tensor.matmul(
                        sc_ps[:, T : T + 128],
                        lhsT=k_sb[:, bi, 128:T],
                        rhs=q_sb[:, bi, 128:T],
                        start=True, stop=True,
                    )
                    expT = ep.tile([128, 3 * 128], BF16, tag="expT")
                    nc.scalar.activation(
                        expT, sc_ps,
                        func=mybir.ActivationFunctionType.Exp,
                        scale=SCALE / (WS * WS),
                    )
                    for qi, quad in enumerate((0, 256)):
                        if mask_eng == 'affine':
                            nc.gpsimd.affine_select(
                                out=expT[:, quad : quad + 128],
                                in_=expT[:, quad : quad + 128],
                                compare_op=mybir.AluOpType.is_ge,
                                fill=0.0, base=0, pattern=[[1, 128]],
                                channel_multiplier=-1,
                            )
                            continue
                        if mask_eng == 'dve':
                            eng = nc.vector
                        elif mask_eng == 'pool':
                            eng = nc.gpsimd
                        else:  # mix: one quadrant each
                            eng = nc.vector if qi == 0 else nc.gpsimd
                        eng.tensor_tensor(
                            expT[:, quad : quad + 128],
                            expT[:, quad : quad + 128],
                            tri,
                            mybir.AluOpType.mult,
                        )
                    expTs.append(expT)
                return (sb, expTs[0], expTs[1], v_sb)

            for k in range(min(pf, NSB)):
                load_sb(k, split=(k == 0))

            pend_sc = None  # superbatch sb-1: awaiting scores/exp/mask
            fin_q = []  # superbatches sb-2, sb-3: awaiting o'/normalize
            for sb in range(NSB):
                if sb + pf < NSB:
                    load_sb(sb + pf, split=(sb + pf == NSB - 1))
                if sb >= 4:
                    nc.sync.dma_start(out[sb - 4], stages.pop(sb - 4))
                xt = xt_tiles[sb]

                # q|k projections for both batches: one 2KB PSUM bank,
                # two accumulation groups of DoubleRow matmuls. scores(sb-1)
                # is emitted BETWEEN the groups so its exp/mask chain starts
                # ~1.5us earlier in the iteration (it is the longest serial
                # chain feeding next iteration's finale).
                qk_ps = qk_psp.tile([128, 2, T], F32, tag="qk")
                n = qk_terms * NCP

                def qk_group(bi):
                    i = 0
                    for xh_, wh_ in TERMS[:qk_terms]:
                        for cp in range(NCP):
                            nc.tensor.matmul(
                                qk_ps[:, bi],
                                lhsT=w_sb[:, 2 * cp : 2 * cp + 2, wh_, 0:128],
                                rhs=xt[:, bi, xh_, 2 * cp : 2 * cp + 2, :],
                                start=(i == 0),
                                stop=(i == n - 1),
                                perf_mode=DR,
                            )
                            i += 1

                qk_group(0)
                qk_group(1)

                # q/k copies first in the DVE/ACT programs: they are the
                # critical arm feeding this superbatch's scores
                q_sb = sbp.tile([64, 2, T], BF16, tag="q_sb")
                k_sb = sbp.tile([64, 2, T], BF16, tag="k_sb")
                nc.vector.tensor_copy(q_sb, qk_ps[0:64])
                nc.scalar.copy(k_sb, qk_ps[64:128])

                # scores(sb-1): operands copied last iteration -> ready now
                if pend_sc is not None:
                    fin_q.append(scores_stage(*pend_sc))
                # finale(sb-2): expT masked last iteration -> ready now
                if len(fin_q) >= 2:
                    final_stage(*fin_q.pop(0))

                # v projections: all four groups in one PSUM bank, single
                # fused ACT copy (emitted before exp in the ACT program)
                v_sb = vp.tile([128, 2, 2, D + 1], BF16, tag="v")
                v_ps = v_psp.tile([128, 2, 2, D], F32, tag="v_ps")
                n = v_terms * NCP
                for bi in range(2):
                    for st in range(2):
                        i = 0
                        for xh_, wh_ in TERMS[:v_terms]:
                            for cp in range(NCP):
                                nc.tensor.matmul(
                                    v_ps[:, bi, st],
                                    lhsT=xt[
                                        :, bi, xh_, 2 * cp : 2 * cp + 2,
                                        st * 128 : (st + 1) * 128,
                                    ],
                                    rhs=w_sb[:, 2 * cp : 2 * cp + 2, wh_, 128:192],
                                    start=(i == 0),
                                    stop=(i == n - 1),
                                    perf_mode=DR,
                                )
                                i += 1
                if vcopy_eng == 'pool':
                    nc.gpsimd.tensor_copy(v_sb[:, :, :, 0:D], v_ps)
                elif vcopy_eng == 'dve':
                    nc.vector.tensor_copy(v_sb[:, :, :, 0:D], v_ps)
                else:
                    nc.scalar.copy(v_sb[:, :, :, 0:D], v_ps)
                nc.gpsimd.memset(v_sb[:, :, :, D : D + 1], WS)

                pend_sc = (sb, q_sb, k_sb, v_sb)

            # drain: scores(15), finals(14..15), stores for sb 12..15
            fin_q.append(scores_stage(*pend_sc))
            nc.sync.dma_start(out[NSB - 4], stages.pop(NSB - 4))
            final_stage(*fin_q.pop(0))
            nc.sync.dma_start(out[NSB - 3], stages.pop(NSB - 3))
            final_stage(*fin_q.pop(0))
            nc.sync.dma_start(out[NSB - 2], stages.pop(NSB - 2))
            last = stages.pop(NSB - 1)
            nc.sync.dma_start(out[NSB - 1][:, 0], last[:, 0])
            nc.sync.dma_start(out[NSB - 1][:, 1], last[:, 1])
    nc.compile()
    return nc


def _pack_inputs(x, Wq, Wk, Wv):
    """Host-side layout/dtype prep: per-core [NSB,128,2,2,CCH,T] fp8 hi/lo x
    and the shared packed weight blob."""
    xt = np.ascontiguousarray(np.transpose(x, (0, 2, 1)))  # [B, C, T] f32
    xh = xt.astype(E4M3)
    xl = (xt - xh.astype(np.float32)).astype(E4M3)
    # [B, C, T] -> [B//2, 2, CCH, 128, T] -> stack hl -> [NSB*8, 128, 2, 2, CCH, T]
    def pack(a):
        return a.reshape(B // 2, 2, CCH, 128, T)
    ph, pl = pack(xh), pack(xl)
    xhl = np.stack([ph, pl], axis=2)  # [B//2, 2, 2, CCH, 128, T]
    xhl = np.ascontiguousarray(xhl.transpose(0, 4, 1, 2, 3, 5))

    wcat = np.concatenate([Wq, Wk, Wv], axis=1) * WS  # [C, 192]
    wh = wcat.astype(E4M3)
    wl = (wcat - wh.astype(np.float32)).astype(E4M3)
    # [C, 192] -> [CCH, 128, 192] -> [128, CCH, 2, 192]
    wall = np.stack(
        [wh.reshape(CCH, 128, 192), wl.reshape(CCH, 128, 192)], axis=2
    ).transpose(1, 0, 2, 3)
    return np.ascontiguousarray(xhl), np.ascontiguousarray(wall)


def kernel(x: np.ndarray, Wq: np.ndarray, Wk: np.ndarray, Wv: np.ndarray) -> np.ndarray:
    global LAST_RESULT
    x = np.asarray(x, dtype=np.float32)
    Wq = np.asarray(Wq, dtype=np.float32)
    Wk = np.asarray(Wk, dtype=np.float32)
    Wv = np.asarray(Wv, dtype=np.float32)

    xhl, wall = _pack_inputs(x, Wq, Wk, Wv)

    nc = _build()
    in_maps = [
        {"xhl": xhl[i * NSB : (i + 1) * NSB], "wall": wall}
        for i in range(NCORES)
    ]
    res = run_bass_kernel_spmd(
        nc, in_maps, core_ids=list(range(NCORES)), trace=TRACE
    )
    LAST_RESULT = res
    # [NSB, 128, 2, 2, D] -> [NSB, 2, 2, 128, D] -> [BPC, T, D]
    outs = [
        np.ascontiguousarray(r["out"].transpose(0, 2, 3, 1, 4))
        .reshape(BPC, T, D)
        .astype(np.float32)
        for r in res.results
    ]
    return np.concatenate(outs, axis=0)


if __name__ == "__main__":
    x = np.random.randn(B, T, C).astype(np.float32)
    Wq = np.random.randn(C, D).astype(np.float32) * (C**-0.5)
    Wk = np.random.randn(C, D).astype(np.float32) * (C**-0.5)
    Wv = np.random.randn(C, D).astype(np.float32) * (C**-0.5)
    o = kernel(x, Wq, Wk, Wv)
    print(o.shape, o.dtype)


# revision 38
# speedup vs baseline: 1.2399x; 1.0088x over previous
"""Single-head causal attention (B=256, T=256, C=1024, D=64) on 8 TRN2 NeuronCores.

Data-parallel over batch (32 batches/core). The schedule is built to be
DMA-bandwidth-bound (the per-core x traffic is the roofline) with every other
engine holding slack:

  * x ships as an fp8-e4m3 hi/lo pair (x = xh + xl, xl the unscaled residual)
    pre-transposed to partition-major [C, T]. Same bytes as bf16, but every
    projection matmul runs in DoubleRow fp8 mode (4x bf16 throughput):
        q|k^T [128,T] = sum of 3 terms  xh@wh + xl@wh + xh@wl   (12 matmuls)
        v     [T,D]   = same 3 terms, xT-stationary             (24 matmuls)
    Weights are host-prescaled by 8 so their fp8 split stays in the normal
    range; the 8x comes out in the softmax scale and the fused denominator.
  * scores^T = k @ q^T in bf16 from a single [128,T] PSUM->SBUF copy (k rows
    sit at partitions 64:128; tile_position=(0,0) passed explicitly).
  * exp via ACT (scale absorbs C**-0.5 / 8**2), causal mask via GpSimd
    affine_select on the two diagonal quadrants only.
  * o' [T, 2, D+1] = e^T.T @ [v | 8] fuses numerator + denominator; DVE
    normalizes into a bf16 2-batch staging tile (host upcasts to f32).
  * DMA: one 1MB load per 2 batches (8KB/partition contiguous), one 64KB
    store per 2 batches, one packed weight load -- 34 DMA instructions total,
    all with >=512B descriptors (no sub-512B latency penalty).
"""

import numpy as np
import ml_dtypes

import concourse.bacc as bacc
import concourse.mybir as mybir
import concourse.tile as tile
from concourse.bass_utils import run_bass_kernel_spmd

B, T, C, D = 256, 256, 1024, 64
NCORES = 8
BPC = B // NCORES  # batches per core
NSB = BPC // 2  # superbatches (2 batches per DMA)
CCH = C // 128  # contraction chunks
NCP = CCH // 2  # chunk pairs (DoubleRow contracts 2 chunks/instruction)
WS = 8.0  # host weight prescale; keeps fp8 weight splits in normal range
SCALE = float(C) ** -0.5

BF16 = mybir.dt.bfloat16
F32 = mybir.dt.float32
F8 = mybir.dt.float8e4
E4M3 = ml_dtypes.float8_e4m3
DR = mybir.MatmulPerfMode.DoubleRow

TRACE = False
LAST_RESULT = None

# (x half, w half) term order: hi@hi, lo@hi, hi@lo
TERMS = ((0, 0), (1, 0), (0, 1))


def _build(pf=2, qk_terms=2, v_terms=3, mask_eng='affine', vcopy_eng='act'):
    nc = bacc.Bacc(
        "TRN2", target_bir_lowering=False, debug=False, num_devices=NCORES
    )
    # [sb, partition, batch-in-sb, hi/lo, chunk, t]
    xhl = nc.dram_tensor("xhl", [NSB, 128, 2, 2, CCH, T], F8, kind="ExternalInput")
    # [partition, chunk, hi/lo, .] : wqk = [Wq|Wk]*8, wv = Wv*8
    wqk_d = nc.dram_tensor("wqk_d", [128, CCH, 2, 128], F8, kind="ExternalInput")
    wv_d = nc.dram_tensor("wv_d", [128, CCH, 2, 64], F8, kind="ExternalInput")
    # [sb, partition, batch-in-sb, t-tile, d]
    out = nc.dram_tensor("out", [NSB, 128, 2, 2, D], BF16, kind="ExternalOutput")

    with tile.TileContext(nc) as tc:
        with (
            tc.tile_pool(name="singles", bufs=1) as singles,
            tc.tile_pool(name="xp", bufs=pf + 1) as xp,
            tc.tile_pool(name="sbp", bufs=3) as sbp,
            tc.tile_pool(name="ep", bufs=8) as ep,
            tc.tile_pool(name="vp", bufs=6) as vp,
            tc.tile_pool(name="stp", bufs=5) as stp,
            tc.tile_pool(name="rp", bufs=2) as rp,
            tc.tile_pool(name="qk_ps", bufs=2, space="PSUM") as qk_psp,
            tc.tile_pool(name="sc_ps", bufs=3, space="PSUM") as sc_psp,
            tc.tile_pool(name="v_ps", bufs=2, space="PSUM") as v_psp,
            tc.tile_pool(name="o_ps", bufs=1, space="PSUM") as o_psp,
        ):
            # qk weights load first: they gate the very first projection
            wqk_sb = singles.tile([128, CCH, 2, 128], F8)
            nc.sync.dma_start(wqk_sb, wqk_d[:])
            wv_sb = singles.tile([128, CCH, 2, D], F8)

            # causal triangle (1 where s <= t within a 128-tile) built once;
            # masking is then a tensor-tensor multiply on any engine
            tri = singles.tile([128, 128], BF16)
            nc.gpsimd.memset(tri, 1.0)
            nc.gpsimd.affine_select(
                out=tri, in_=tri,
                compare_op=mybir.AluOpType.is_ge,
                fill=0.0, base=0, pattern=[[1, 128]], channel_multiplier=-1,
            )

            xt_tiles = {}

            def load_sb(k, split=False):
                t = xp.tile([128, 2, 2, CCH, T], F8, tag="xt")
                if split:
                    # per-batch halves: first batch's data (and compute)
                    # lands ~1.5us earlier at the pipeline head/tail
                    nc.sync.dma_start(t[:, 0], xhl[k][:, 0])
                    nc.sync.dma_start(t[:, 1], xhl[k][:, 1])
                else:
                    nc.sync.dma_start(t, xhl[k])
                xt_tiles[k] = t

            stages = {}

            def final_stage(sb, expT0, expT1, v_sb):
                """o' matmuls + softmax normalization for superbatch sb
                (emitted two superbatches late)."""
                o2 = o_psp.tile([128, 2, 2, D + 1], F32, tag="o_ps")
                for bi, expT in ((0, expT0), (1, expT1)):
                    nc.tensor.matmul(
                        o2[:, bi, 0], lhsT=expT[:, 0:128], rhs=v_sb[:, bi, 0],
                        start=True, stop=True,
                    )
                    nc.tensor.matmul(
                        o2[:, bi, 1], lhsT=expT[:, 128:256], rhs=v_sb[:, bi, 0],
                        start=True, stop=False,
                    )
                    nc.tensor.matmul(
                        o2[:, bi, 1], lhsT=expT[:, 256:384], rhs=v_sb[:, bi, 1],
                        start=False, stop=True,
                    )
                stages[sb] = stp.tile(
                    [128, 2, 2, D], BF16, tag="stage", name="stage"
                )
                # recip to SBUF first: engines may read only ONE PSUM
                # operand per instruction, and Pool can't read PSUM at all
                recip = rp.tile([128, 2, 2], F32, tag="recip")
                nc.vector.reciprocal(recip, o2[:, :, :, D])
                nc.vector.tensor_tensor(
                    stages[sb],
                    o2[:, :, :, 0:D],
                    recip[:, :, :, None].to_broadcast((128, 2, 2, D)),
                    mybir.AluOpType.mult,
                )

            def scores_stage(sb, q_sb, k_sb, v_sb):
                """scores^T + exp + causal mask for both batches of sb
                (emitted one superbatch late)."""
                expTs = []
                for bi in range(2):
                    # scores^T packed [128, 384]: cols 0:256 = (s<128, all t),
                    # 256:384 = (s>=128, t>=128); (s>=128, t<128) fully masked
                    sc_ps = sc_psp.tile([128, 3 * 128], F32, tag="sc")
                    nc.tensor.matmul(
                        sc_ps[:, 0:T],
                        lhsT=k_sb[:, bi, 0:128],
                        rhs=q_sb[:, bi],
                        start=True, stop=True,
                    )
                    nc.tensor.matmul(
                        sc_ps[:, T : T + 128],
                        lhsT=k_sb[:, bi, 128:T],
                        rhs=q_sb[:, bi, 128:T],
                        start=True, stop=True,
                    )
                    expT = ep.tile([128, 3 * 128], BF16, tag="expT")
                    nc.scalar.activation(
                        expT, sc_ps,
                        func=mybir.ActivationFunctionType.Exp,
                        scale=SCALE / (WS * WS),
                    )
                    for qi, quad in enumerate((0, 256)):
                        if mask_eng == 'affine':
                            nc.gpsimd.affine_select(
                                out=expT[:, quad : quad + 128],
                                in_=expT[:, quad : quad + 128],
                                compare_op=mybir.AluOpType.is_ge,
                                fill=0.0, base=0, pattern=[[1, 128]],
                                channel_multiplier=-1,
                            )
                            continue
                        if mask_eng == 'dve':
                            eng = nc.vector
                        elif mask_eng == 'pool':
                            eng = nc.gpsimd
                        else:  # mix: one quadrant each
                            eng = nc.vector if qi == 0 else nc.gpsimd
                        eng.tensor_tensor(
                            expT[:, quad : quad + 128],
                            expT[:, quad : quad + 128],
                            tri,
                            mybir.AluOpType.mult,
                        )
                    expTs.append(expT)
                return (sb, expTs[0], expTs[1], v_sb)

            load_sb(0, split=True)
            nc.sync.dma_start(wv_sb, wv_d[:])
            for k in range(1, min(pf, NSB)):
                load_sb(k)

            pend_sc = None  # superbatch sb-1: awaiting scores/exp/mask
            fin_q = []  # superbatches sb-2, sb-3: awaiting o'/normalize
            for sb in range(NSB):
                if sb + pf < NSB:
                    load_sb(sb + pf, split=(sb + pf == NSB - 1))
                if sb >= 4:
                    nc.sync.dma_start(out[sb - 4], stages.pop(sb - 4))
                xt = xt_tiles[sb]

                # q|k projections for both batches: one 2KB PSUM bank,
                # two accumulation groups of DoubleRow matmuls. scores(sb-1)
                # is emitted BETWEEN the groups so its exp/mask chain starts
                # ~1.5us earlier in the iteration (it is the longest serial
                # chain feeding next iteration's finale).
                qk_ps = qk_psp.tile([128, 2, T], F32, tag="qk")
                n = qk_terms * NCP

                def qk_group(bi):
                    i = 0
                    for xh_, wh_ in TERMS[:qk_terms]:
                        for cp in range(NCP):
                            nc.tensor.matmul(
                                qk_ps[:, bi],
                                lhsT=wqk_sb[:, 2 * cp : 2 * cp + 2, wh_],
                                rhs=xt[:, bi, xh_, 2 * cp : 2 * cp + 2, :],
                                start=(i == 0),
                                stop=(i == n - 1),
                                perf_mode=DR,
                            )
                            i += 1

                qk_group(0)
                qk_group(1)

                # q/k copies first in the DVE/ACT programs: they are the
                # critical arm feeding this superbatch's scores
                q_sb = sbp.tile([64, 2, T], BF16, tag="q_sb")
                k_sb = sbp.tile([64, 2, T], BF16, tag="k_sb")
                nc.vector.tensor_copy(q_sb, qk_ps[0:64])
                nc.scalar.copy(k_sb, qk_ps[64:128])


                # v projections: all four groups in one PSUM bank, single
                # fused ACT copy (emitted before exp in the ACT program)
                v_sb = vp.tile([128, 2, 2, D + 1], BF16, tag="v")
                v_ps = v_psp.tile([128, 2, 2, D], F32, tag="v_ps")
                n = v_terms * NCP
                for bi in range(2):
                    for st in range(2):
                        i = 0
                        for xh_, wh_ in TERMS[:v_terms]:
                            for cp in range(NCP):
                                nc.tensor.matmul(
                                    v_ps[:, bi, st],
                                    lhsT=xt[
                                        :, bi, xh_, 2 * cp : 2 * cp + 2,
                                        st * 128 : (st + 1) * 128,
                                    ],
                                    rhs=wv_sb[:, 2 * cp : 2 * cp + 2, wh_],
                                    start=(i == 0),
                                    stop=(i == n - 1),
                                    perf_mode=DR,
                                )
                                i += 1
                if vcopy_eng == 'pool':
                    nc.gpsimd.tensor_copy(v_sb[:, :, :, 0:D], v_ps)
                elif vcopy_eng == 'dve':
                    nc.vector.tensor_copy(v_sb[:, :, :, 0:D], v_ps)
                else:
                    nc.scalar.copy(v_sb[:, :, :, 0:D], v_ps)
                nc.gpsimd.memset(v_sb[:, :, :, D : D + 1], WS)

                # scores(sb-1): operands copied last iteration -> ready now
                if pend_sc is not None:
                    fin_q.append(scores_stage(*pend_sc))
                # finale(sb-2): expT masked last iteration -> ready now
                if len(fin_q) >= 2:
                    final_stage(*fin_q.pop(0))

                pend_sc = (sb, q_sb, k_sb, v_sb)

            # drain: scores(15), finals(14..15), stores for sb 12..15
            fin_q.append(scores_stage(*pend_sc))
            nc.sync.dma_start(out[NSB - 4], stages.pop(NSB - 4))
            final_stage(*fin_q.pop(0))
            nc.sync.dma_start(out[NSB - 3], stages.pop(NSB - 3))
            final_stage(*fin_q.pop(0))
            nc.sync.dma_start(out[NSB - 2], stages.pop(NSB - 2))
            last = stages.pop(NSB - 1)
            nc.sync.dma_start(out[NSB - 1][:, 0], last[:, 0])
            nc.sync.dma_start(out[NSB - 1][:, 1], last[:, 1])
    nc.compile()
    return nc


def _pack_inputs(x, Wq, Wk, Wv):
    """Host-side layout/dtype prep: per-core [NSB,128,2,2,CCH,T] fp8 hi/lo x
    and the shared packed weight blob."""
    xt = np.ascontiguousarray(np.transpose(x, (0, 2, 1)))  # [B, C, T] f32
    xh = xt.astype(E4M3)
    xl = (xt - xh.astype(np.float32)).astype(E4M3)
    # [B, C, T] -> [B//2, 2, CCH, 128, T] -> stack hl -> [NSB*8, 128, 2, 2, CCH, T]
    def pack(a):
        return a.reshape(B // 2, 2, CCH, 128, T)
    ph, pl = pack(xh), pack(xl)
    xhl = np.stack([ph, pl], axis=2)  # [B//2, 2, 2, CCH, 128, T]
    xhl = np.ascontiguousarray(xhl.transpose(0, 4, 1, 2, 3, 5))

    def pack_w(W, m):
        w8 = W * WS
        wh = w8.astype(E4M3)
        wl = (w8 - wh.astype(np.float32)).astype(E4M3)
        return np.ascontiguousarray(
            np.stack(
                [wh.reshape(CCH, 128, m), wl.reshape(CCH, 128, m)], axis=2
            ).transpose(1, 0, 2, 3)
        )

    wqk = pack_w(np.concatenate([Wq, Wk], axis=1), 128)
    wv = pack_w(Wv, D)
    return np.ascontiguousarray(xhl), wqk, wv


def kernel(x: np.ndarray, Wq: np.ndarray, Wk: np.ndarray, Wv: np.ndarray) -> np.ndarray:
    global LAST_RESULT
    x = np.asarray(x, dtype=np.float32)
    Wq = np.asarray(Wq, dtype=np.float32)
    Wk = np.asarray(Wk, dtype=np.float32)
    Wv = np.asarray(Wv, dtype=np.float32)

    xhl, wqk, wv = _pack_inputs(x, Wq, Wk, Wv)

    nc = _build()
    in_maps = [
        {"xhl": xhl[i * NSB : (i + 1) * NSB], "wqk_d": wqk, "wv_d": wv}
        for i in range(NCORES)
    ]
    res = run_bass_kernel_spmd(
        nc, in_maps, core_ids=list(range(NCORES)), trace=TRACE
    )
    LAST_RESULT = res
    # [NSB, 128, 2, 2, D] -> [NSB, 2, 2, 128, D] -> [BPC, T, D]
    outs = [
        np.ascontiguousarray(r["out"].transpose(0, 2, 3, 1, 4))
        .reshape(BPC, T, D)
        .astype(np.float32)
        for r in res.results
    ]
    return np.concatenate(outs, axis=0)


if __name__ == "__main__":
    x = np.random.randn(B, T, C).astype(np.float32)
    Wq = np.random.randn(C, D).astype(np.float32) * (C**-0.5)
    Wk = np.random.randn(C, D).astype(np.float32) * (C**-0.5)
    Wv = np.random.randn(C, D).astype(np.float32) * (C**-0.5)
    o = kernel(x, Wq, Wk, Wv)
    print(o.shape, o.dtype)
